# revision 8
# baseline (speedup 1.0000x reference)
"""MixedScore MultiHeadAttention Trainium2 kernel.

Sharding: 8 cores, core i handles batch b = i // 2 and heads
[ (i % 2) * 8, (i % 2) * 8 + 8 ) -- 8 (b, h) pairs per core, D_TM[b] shared.

Per-core pipeline (per head, per 128-row t-tile):
  1. QK^T on TensorE (f32r operands, K zero-padded to 128).
  2. dot rows regrouped with D rows into 16-row stack slots (8 t-rows of dot
     + 8 t-rows of D), 8 slots packed per 128 partitions, via SBUF->SBUF DMA.
  3. One K=128 expansion matmul per group g with a host-built selector
     (nonzero only at the group's 16 rows) produces
     Z[p = t_sub*16 + c, m] = w1[h,0,c]/8 * dot + w1[h,1,c] * D.
  4. relu(Z + b1) split between DVE (tensor_scalar add+max) and ACT
     (activation Relu with per-partition bias).
  5. 16 accumulating reduce matmuls (w2 selector = column-shifted slices of
     one base matrix) rebuild mixed[128 t, 512 m] in PSUM.
  6. Softmax: DVE rowmax (negated), ACT fused exp+rowsum, DVE reciprocal.
  7. P^T via TensorE transpose; AV matmul accumulated over 4 m-chunks;
     output scaled by 1/rowsum on ACT and DMA'd out.
mix2_bias is dropped (softmax shift invariance).
"""

import numpy as np

try:
    import concourse.bass as bass
except ImportError:  # pragma: no cover
    import sys

    sys.path.insert(0, "/opt/trn_rl_repo")
    import concourse.bass as bass

import concourse.mybir as mybir
from concourse import bacc, bass_utils
from concourse.masks import make_identity
from concourse.tile import TileContext

F32 = mybir.dt.float32
F32R = mybir.dt.float32r
MMDT = F32R  # matmul operand dtype; set to F32 for fallback
AF = mybir.ActivationFunctionType
OP = mybir.AluOpType

B, H, T, M, DQK, HID = 4, 16, 512, 512, 64, 16
NHEADS = 8          # heads per core
NTT = T // 128      # t-tiles per head
NG = 16             # 8-row groups per t-tile
GSZ = 8             # t-rows per group
MD = 192            # m-split: DVE gets [0, MD), ACT gets [MD, 512)

_CACHED = {}





def build_bass():
    nc = bacc.Bacc(None, target_bir_lowering=False)
    q = nc.dram_tensor("q_s", [NHEADS, T, DQK], F32, kind="ExternalInput")
    k = nc.dram_tensor("k_s", [NHEADS, M, DQK], F32, kind="ExternalInput")
    v = nc.dram_tensor("v_s", [NHEADS, M, DQK], MMDT, kind="ExternalInput")
    dd = nc.dram_tensor("D_s", [T, M], MMDT, kind="ExternalInput")
    esel = nc.dram_tensor("exp_sel", [NHEADS, 8, 128, 128], MMDT, kind="ExternalInput")
    bv = nc.dram_tensor("bias_v", [NHEADS, 128, 1], F32, kind="ExternalInput")
    rbase = nc.dram_tensor("red_base", [NHEADS, 128, 248], MMDT, kind="ExternalInput")
    o = nc.dram_tensor("out_s", [T, NHEADS * DQK], F32, kind="ExternalOutput")

    with TileContext(nc) as tc:
        with (
            tc.tile_pool(name="const", bufs=1) as const,
            tc.tile_pool(name="stackp", bufs=1) as stackp,
            tc.tile_pool(name="hwork", bufs=2) as hwork,
            tc.tile_pool(name="dwork", bufs=3) as dwork,
            tc.tile_pool(name="rwork", bufs=4) as rwork,
            tc.tile_pool(name="pwork", bufs=2) as pwork,
            tc.tile_pool(name="small", bufs=6) as small,
            tc.tile_pool(name="outp", bufs=3) as outp,
            tc.tile_pool(name="psA", bufs=1, space="PSUM") as psA,
            tc.tile_pool(name="psZ", bufs=2, space="PSUM") as psZ,
            tc.tile_pool(name="psM", bufs=1, space="PSUM") as psM,
            tc.tile_pool(name="psQT", bufs=1, space="PSUM") as psQT,
            tc.tile_pool(name="psPT", bufs=1, space="PSUM") as psPT,
            tc.tile_pool(name="psAV", bufs=1, space="PSUM") as psAV,
        ):
            identity = const.tile([128, 128], F32)
            make_identity(nc, identity[:, :])

            esel_sb = const.tile([128, NHEADS, 8, 128], MMDT)
            nc.sync.dma_start(out=esel_sb[:, :, :, :], in_=esel.rearrange("h j r c -> r h j c"))
            bias_sb = const.tile([128, NHEADS], F32)
            nc.sync.dma_start(out=bias_sb[:, :], in_=bv.rearrange("h p one -> p (h one)"))
            rbase_sb = const.tile([128, NHEADS, 248], MMDT)
            nc.sync.dma_start(out=rbase_sb[:, :, :], in_=rbase.rearrange("h p y -> p h y"))

            # Stack tiles: slot (tt, g) at partitions [16*(g%8), +16), free
            # slot (tt, g//8).  Rows 0..7 = dot rows (filled per head), rows
            # 8..15 = D rows (filled once here).
            stacks = stackp.tile([128, NTT, 2, 512], MMDT)
            for tt in range(NTT):
                for g in range(NG):
                    nc.sync.dma_start(
                        out=stacks[16 * (g % 8) + 8 : 16 * (g % 8) + 16, tt, g // 8, :],
                        in_=dd[tt * 128 + 8 * g : tt * 128 + 8 * g + 8, :],
                    )

            for h in range(NHEADS):
                q_sb = hwork.tile([128, NTT, 128], F32, tag="qsb")
                k_sb = hwork.tile([128, NTT, 128], F32, tag="ksb")
                v_sb = hwork.tile([128, NTT, DQK], MMDT, tag="vsb")
                nc.vector.memset(q_sb[:, :, :], 0.0)
                nc.vector.memset(k_sb[:, :, :], 0.0)
                nc.sync.dma_start(out=q_sb[:, :, 0:DQK], in_=q[h].rearrange("(c p) d -> p c d", p=128))
                nc.sync.dma_start(out=k_sb[:, :, 0:DQK], in_=k[h].rearrange("(c p) d -> p c d", p=128))
                nc.sync.dma_start(out=v_sb[:, :, :], in_=v[h].rearrange("(c p) d -> p c d", p=128))

                # qT/kT: [128, 512] with rows 64..127 zero (K padding for QK).
                qT = hwork.tile([128, 512], MMDT, tag="qT")
                kT = hwork.tile([128, 512], MMDT, tag="kT")
                qt_ps = psQT.tile([128, 512], F32)
                for c in range(4):
                    nc.tensor.transpose(qt_ps[:, c * 128 : (c + 1) * 128], q_sb[:, c, :], identity[:, :])
                nc.scalar.copy(qT[:, :], qt_ps[:, :])
                kt_ps = psQT.tile([128, 512], F32)
                for c in range(4):
                    nc.tensor.transpose(kt_ps[:, c * 128 : (c + 1) * 128], k_sb[:, c, :], identity[:, :])
                nc.scalar.copy(kT[:, :], kt_ps[:, :])

                for tt in range(NTT):
                    dot_ps = psA.tile([128, 512], F32)
                    nc.tensor.matmul(
                        dot_ps[:, :], (qT[:, tt * 128 : (tt + 1) * 128]), (kT[:, :]),
                        start=True, stop=True,
                    )
                    dot_sb = dwork.tile([128, 512], MMDT)
                    nc.vector.tensor_copy(dot_sb[:, :], dot_ps[:, :])
                    for g in range(NG):
                        nc.sync.dma_start(
                            out=stacks[16 * (g % 8) : 16 * (g % 8) + 8, tt, g // 8, :],
                            in_=dot_sb[8 * g : 8 * g + 8, :],
                        )

                    mixed_ps = psM.tile([128, 512], F32)
                    for g in range(NG):
                        z_ps = psZ.tile([128, 512], F32)
                        nc.tensor.matmul(
                            z_ps[:, :],
                            (esel_sb[:, h, g % 8, :]),
                            (stacks[:, tt, g // 8, :]),
                            start=True, stop=True,
                        )
                        rt = rwork.tile([128, 512], MMDT)
                        nc.vector.tensor_scalar(
                            rt[:, 0:MD], z_ps[:, 0:MD],
                            bias_sb[:, h : h + 1], 0.0, op0=OP.add, op1=OP.max,
                        )
                        nc.scalar.activation(
                            rt[:, MD:512], z_ps[:, MD:512], AF.Relu,
                            bias=bias_sb[:, h : h + 1], scale=1.0,
                        )
                        nc.tensor.matmul(
                            mixed_ps[:, :],
                            (rbase_sb[:, h, 120 - 8 * g : 248 - 8 * g]),
                            (rt[:, :]),
                            start=(g == 0), stop=(g == NG - 1),
                        )

                    negmax = small.tile([128, 1], F32, tag="negmax")
                    nc.vector.tensor_reduce(
                        negmax[:, :], mixed_ps[:, :], axis=mybir.AxisListType.X,
                        op=OP.max, negate=True,
                    )
                    pP = pwork.tile([128, 512], F32, tag="pP")
                    rowsum = small.tile([128, 1], F32, tag="rowsum")
                    nc.scalar.activation(
                        pP[:, :], mixed_ps[:, :], AF.Exp,
                        bias=negmax[:, :], scale=1.0, accum_out=rowsum[:, :],
                    )
                    rinv = small.tile([128, 1], F32, tag="rinv")
                    nc.vector.reciprocal(rinv[:, :], rowsum[:, :])

                    pt_ps = psPT.tile([128, 512], F32)
                    for c in range(4):
                        nc.tensor.transpose(
                            pt_ps[:, c * 128 : (c + 1) * 128], pP[:, c * 128 : (c + 1) * 128],
                            identity[:, :],
                        )
                    pT = pwork.tile([128, 512], MMDT, tag="pT")
                    nc.scalar.copy(pT[:, :], pt_ps[:, :])

                    av_ps = psAV.tile([128, DQK], F32)
                    for c in range(4):
                        nc.tensor.matmul(
                            av_ps[:, :], (pT[:, c * 128 : (c + 1) * 128]), (v_sb[:, c, :]),
                            start=(c == 0), stop=(c == 3),
                        )
                    o_sb = outp.tile([128, DQK], F32)
                    nc.scalar.activation(o_sb[:, :], av_ps[:, :], AF.Copy, scale=rinv[:, :])
                    nc.sync.dma_start(
                        out=o[tt * 128 : (tt + 1) * 128, h * DQK : (h + 1) * DQK],
                        in_=o_sb[:, :],
                    )
    nc.compile()
    return nc


def host_prepare(q, k, v, D_TM, mix1_weight, mix1_bias, mix2_weight, mix2_bias):
    """Build per-core input maps (sharding + host-computed selectors)."""
    q = np.ascontiguousarray(np.asarray(q, dtype=np.float32))
    k = np.ascontiguousarray(np.asarray(k, dtype=np.float32))
    v = np.ascontiguousarray(np.asarray(v, dtype=np.float32))
    D_TM = np.ascontiguousarray(np.asarray(D_TM, dtype=np.float32))
    w1 = np.asarray(mix1_weight, dtype=np.float32)
    b1 = np.asarray(mix1_bias, dtype=np.float32)
    w2 = np.asarray(mix2_weight, dtype=np.float32)

    in_maps = []
    for core in range(8):
        b = core // 2
        hs = (core % 2) * NHEADS
        esel = np.zeros((NHEADS, 8, 128, 128), np.float32)
        bvv = np.zeros((NHEADS, 128, 1), np.float32)
        rbase = np.zeros((NHEADS, 128, 248), np.float32)
        for j in range(NHEADS):
            h = hs + j
            for ts in range(8):
                p0 = ts * 16
                for jj in range(8):
                    esel[j, jj, 16 * jj + ts, p0 : p0 + 16] = w1[h, 0, :] / 8.0
                    esel[j, jj, 16 * jj + 8 + ts, p0 : p0 + 16] = w1[h, 1, :]
                bvv[j, p0 : p0 + 16, 0] = b1[h, :]
                rbase[j, p0 : p0 + 16, ts + 120] = w2[h, :, 0]
        in_maps.append(
            {
                "q_s": np.ascontiguousarray(q[b, hs : hs + NHEADS]),
                "k_s": np.ascontiguousarray(k[b, hs : hs + NHEADS]),
                "v_s": np.ascontiguousarray(v[b, hs : hs + NHEADS]),
                "D_s": np.ascontiguousarray(D_TM[b]),
                "exp_sel": esel,
                "bias_v": bvv,
                "red_base": rbase,
            }
        )
    return in_maps


def assemble(results):
    out = np.zeros((B, T, H * DQK), np.float32)
    for core in range(8):
        b = core // 2
        hs = (core % 2) * NHEADS
        out[b, :, hs * DQK : (hs + NHEADS) * DQK] = results[core]["out_s"]
    return out


def kernel(**inputs):
    in_maps = host_prepare(**inputs)
    if "nc" not in _CACHED:
        _CACHED["nc"] = build_bass()
    nc = _CACHED["nc"]
    res = bass_utils.run_bass_kernel_spmd(nc, in_maps, core_ids=list(range(8)))
    return assemble(res.results)
